# revision 12
# baseline (speedup 1.0000x reference)
"""Multi-head causal attention (B=4, T=2048, D=512, H=8) on 8 TRN2 NeuronCores.

Sharding: core c handles batch b = c//2 and head-group hg = c%2 (4 heads,
256 output dims).  No collectives needed — 8 fully independent problems.

Per-core algorithm (matmul inputs bf16, O^T accumulation f32 in PSUM):
  - host passes x^T (D,T) and W^T slices (D, 256) in bf16 + a [128,128]
    triangular causal mask
  - Q^T,K^T projections:  qT[dh2,T] = W2h @ xT, two heads stacked per tile
  - V projection into augmented-V tiles [k-tile 128, 65] (ones column
    appended -> the O^T matmul also produces the softmax denominator row)
  - flash-style: S^T[k,q] = K^T.T @ Q^T per (k-tile, q-block), exp via ACT
    (scale=1/8 folded in; no max subtraction: |scores| < ~4).  Causal:
    diagonal k-tiles only compute q >= k-tile start, triangle-mask multiply
    on boundary blocks.
  - O^T accumulated in PSUM over k-tiles; numerator+denominator cast to
    bf16, DMA-transposed to natural layout, divide on DVE -> [T,256] -> out

Scheduling (program order == Tile priority): minimal projection prologue
(kT chunk 0 + qT chunk 3 of group 0), then attention head-pairs with all
remaining projection/V units woven between score batches as PE filler.
Heads of a pair alternate PE row-groups (partition offsets 0/64) so
LDWEIGHTS can pull ahead.  qb runs DESCENDING so attention opens with its
PE-densest stretch — the PE clock (HAM) never throttles down; re-warming
from cold needs a fully-busy 3.4us window sparse-qb units can't provide.
"""

import numpy as np
import ml_dtypes

T = 2048
D = 512
HG = 4  # heads per core
DH = 64
OUTW = HG * DH  # 256
QB = 512  # q block (columns of S^T tiles)
NQB = T // QB  # 4
NKT = T // 128  # 16 k-tiles
N_CORES = 8

_CACHE = {}


def _build_nc():
    import concourse.bacc as bacc
    import concourse.tile as tile
    import concourse.mybir as mybir
    from contextlib import ExitStack

    fp32 = mybir.dt.float32
    bf16 = mybir.dt.bfloat16
    EXP = mybir.ActivationFunctionType.Exp

    nc = bacc.Bacc(None, target_bir_lowering=False)

    xt_d = nc.declare_dram_parameter("xt", [D, T], bf16, isOutput=False)
    wqt_d = nc.declare_dram_parameter("wqt", [D, OUTW], bf16, isOutput=False)
    wkt_d = nc.declare_dram_parameter("wkt", [D, OUTW], bf16, isOutput=False)
    wvt_d = nc.declare_dram_parameter("wvt", [D, OUTW], bf16, isOutput=False)
    cmask_d = nc.declare_dram_parameter("cmask", [128, 128], bf16, isOutput=False)
    out_d = nc.declare_dram_parameter("out", [T, OUTW], fp32, isOutput=True)

    with tile.TileContext(nc) as tc, ExitStack() as ctx:
        const = ctx.enter_context(tc.tile_pool(name="const", bufs=1))
        ps_s = ctx.enter_context(tc.tile_pool(name="ps_s", bufs=2, space="PSUM"))
        pt_pool = ctx.enter_context(tc.tile_pool(name="pt", bufs=4))
        osb_pool = ctx.enter_context(tc.tile_pool(name="osb", bufs=3))
        tpb_pool = ctx.enter_context(tc.tile_pool(name="tpb", bufs=6))
        rec_pool = ctx.enter_context(tc.tile_pool(name="rec", bufs=4))

        # ---- input loads: weights on sync, x on scalar (parallel queues)
        def load4(dram, name, width, eng):
            ts = []
            for c in range(4):
                t = const.tile([128, width], bf16, tag=f"{name}{c}", name=f"{name}{c}")
                eng.dma_start(out=t[:], in_=dram[c * 128:(c + 1) * 128, :])
                ts.append(t)
            return ts

        wkT = load4(wkt_d, "wkT", OUTW, nc.sync)
        wqT = load4(wqt_d, "wqT", OUTW, nc.sync)
        xT = load4(xt_d, "xT", T, nc.scalar)
        wvT = load4(wvt_d, "wvT", OUTW, nc.sync)

        mask_sb = const.tile([128, 128], bf16, name="mask_sb")
        nc.sync.dma_start(out=mask_sb[:], in_=cmask_d[:])

        # ---- persistent SBUF tensors ----
        qT = [const.tile([128, T], bf16, tag=f"qT{g}", name=f"qT{g}") for g in range(2)]
        kT = [const.tile([128, T], bf16, tag=f"kT{g}", name=f"kT{g}") for g in range(2)]
        vaug = const.tile([128, NKT, HG, 65], bf16, name="vaug")
        nc.vector.memset(vaug[:, :, :, 64:65], 1.0)
        out_sb = const.tile([128, NQB * 4, OUTW], fp32, tag="out_sb", name="out_sb")

        def proj_qk(dst, wt, g, qb4):
            ps = ps_s.tile([128, QB], fp32, tag="ps", name="ps")
            for c in range(4):
                nc.tensor.matmul(
                    ps[:],
                    wt[c][:, g * 128:(g + 1) * 128],
                    xT[c][:, qb4 * QB:(qb4 + 1) * QB],
                    start=(c == 0),
                    stop=(c == 3),
                )
            nc.vector.tensor_copy(dst[g][:, qb4 * QB:(qb4 + 1) * QB], ps[:])

        def proj_v(tt):
            ps = ps_s.tile([128, OUTW], fp32, tag="ps", name="ps")
            for c in range(4):
                nc.tensor.matmul(
                    ps[:],
                    xT[c][:, tt * 128:(tt + 1) * 128],
                    wvT[c][:, 0:OUTW],
                    start=(c == 0),
                    stop=(c == 3),
                )
            nc.vector.tensor_copy(
                vaug[:, tt, :, 0:64],
                ps[:].rearrange("p (h d) -> p h d", h=HG),
            )

        def attn_pair(qb, g, filler=None):
            """Attention for heads 2g, 2g+1 over q-block qb, batch-interleaved.
            filler(i) is called once per score-batch to weave in PE work."""
            heads = (2 * g, 2 * g + 1)
            ots = {}
            for h in heads:
                ots[h] = ps_s.tile([128, QB], fp32, tag="ot", name="ot")
            last_kt = qb * 4 + 3

            def score_mm(h, st_ap, kt, q0, width):
                po = 64 * (h % 2)
                nc.tensor.matmul(
                    st_ap,
                    kT[g][po:po + 64, kt * 128:(kt + 1) * 128],
                    qT[g][po:po + 64, qb * QB + q0: qb * QB + q0 + width],
                    start=True,
                    stop=True,
                )

            def ot_mm(h, kt, pt_ap, q0, width):
                nc.tensor.matmul(
                    ots[h][0:65, q0:q0 + width],
                    vaug[:, kt, h, :],
                    pt_ap,
                    start=(kt == 0),
                    stop=(kt == last_kt),
                )

            # batch list: off-diagonal pairs, then two diagonal pairs with
            # causal restriction (widths 512/384 and 256/128)
            batches = [((kt0, 0, QB), (kt0 + 1, 0, QB))
                       for kt0 in range(0, qb * 4, 2)]
            batches.append(((qb * 4 + 0, 0, QB), (qb * 4 + 1, 128, QB - 128)))
            batches.append(((qb * 4 + 2, 256, QB - 256), (qb * 4 + 3, 384, QB - 384)))
            ndiag = 2

            for i, ((kta, qa, wa), (ktb, qbk, wb)) in enumerate(batches):
                diag = i >= len(batches) - ndiag
                sts, pts = {}, {}
                for h in heads:
                    st = ps_s.tile([128, 2 * QB], fp32, tag="st", name="st")
                    score_mm(h, st[:, 0:wa], kta, qa, wa)
                    score_mm(h, st[:, wa:wa + wb], ktb, qbk, wb)
                    sts[h] = st
                if filler:
                    filler(i)
                for h in heads:
                    pt = pt_pool.tile([128, 2 * QB], bf16, tag="pt", name="pt")
                    nc.scalar.activation(
                        pt[:, 0:wa + wb], sts[h][:, 0:wa + wb],
                        func=EXP, scale=0.125,
                    )
                    if diag:
                        nc.vector.tensor_mul(
                            pt[:, 0:128], pt[:, 0:128], mask_sb[:]
                        )
                        nc.vector.tensor_mul(
                            pt[:, wa:wa + 128], pt[:, wa:wa + 128], mask_sb[:]
                        )
                    pts[h] = pt
                for h in heads:
                    ot_mm(h, kta, pts[h][:, 0:wa], qa, wa)
                    ot_mm(h, ktb, pts[h][:, wa:wa + wb], qbk, wb)

            # normalize via bf16 DMA-transpose (PE stays free):
            # osb rows 0:64 numerator, row 64 denominator, rows 65:80 pad
            for h in heads:
                osb = osb_pool.tile([80, QB], bf16, tag="osb", name="osb")
                nc.vector.tensor_copy(osb[0:65, :], ots[h][0:65, :])
                for j4 in range(4):
                    tpb = tpb_pool.tile([128, 80], bf16, tag="tpb", name="tpb")
                    nc.sync.dma_start(
                        out=tpb[:],
                        in_=osb[:, j4 * 128:(j4 + 1) * 128],
                        transpose=True,
                    )
                    rec = rec_pool.tile([128, 1], fp32, tag="rec", name="rec")
                    nc.vector.reciprocal(rec[:], tpb[:, 64:65])
                    nc.vector.tensor_scalar_mul(
                        out_sb[:, qb * 4 + j4, h * 64:(h + 1) * 64],
                        tpb[:, 0:64],
                        rec[:],
                    )

        def stream_out(qb):
            for j4 in range(4):
                tt = qb * 4 + j4
                nc.sync.dma_start(
                    out=out_d[tt * 128:(tt + 1) * 128, :], in_=out_sb[:, tt, :]
                )

        # ---- schedule ----
        # minimal prologue: just what pair(3, g0)'s first score batch needs
        proj_qk(kT, wkT, 0, 0)
        proj_qk(qT, wqT, 0, 3)

        # pair(3, g0): weave V units (ot batch i needs vaug 2i, 2i+1), the
        # remaining g0 K chunks (scores batch i reads kT chunk i//2), and the
        # remaining g0 Q chunks (for later qbs) into the 8 filler slots
        g0_fill = {
            0: [("v", 0), ("v", 1), ("k", 0, 1)],
            1: [("v", 2), ("v", 3), ("k", 0, 2)],
            2: [("v", 4), ("v", 5), ("k", 0, 3)],
            3: [("v", 6), ("v", 7), ("q", 0, 2)],
            4: [("v", 8), ("v", 9), ("q", 0, 1)],
            5: [("v", 10), ("v", 11), ("q", 0, 0)],
            6: [("v", 12), ("v", 13)],
            7: [("v", 14), ("v", 15)],
        }
        # pair(3, g1): weave the g1 projections into its own filler slots;
        # prologue chunks (kT g1 c0, qT g1 c3) ride in pair(3, g0) slots 6/7
        g0_fill[6].append(("k", 1, 0))
        g0_fill[7].append(("q", 1, 3))
        g1_fill = {
            0: [("k", 1, 1)],
            1: [("k", 1, 2)],
            2: [("k", 1, 3)],
            3: [("q", 1, 2)],
            4: [("q", 1, 1)],
            5: [("q", 1, 0)],
        }

        def run_fill(plan, i):
            for item in plan.get(i, ()):
                if item[0] == "v":
                    proj_v(item[1])
                elif item[0] == "k":
                    proj_qk(kT, wkT, item[1], item[2])
                else:
                    proj_qk(qT, wqT, item[1], item[2])

        attn_pair(3, 0, filler=lambda i: run_fill(g0_fill, i))
        attn_pair(3, 1, filler=lambda i: run_fill(g1_fill, i))
        stream_out(3)
        for qb in (2, 1, 0):
            for g in (0, 1):
                attn_pair(qb, g)
            stream_out(qb)

    nc.finalize()
    return nc


def _get_nc():
    if "nc" not in _CACHE:
        _CACHE["nc"] = _build_nc()
    return _CACHE["nc"]


def _make_cmask():
    # triangle: mask[p, f] = 1.0 iff p <= f
    p = np.arange(128)[:, None]
    f = np.arange(128)[None, :]
    return (p <= f).astype(ml_dtypes.bfloat16)


def _make_in_maps(x, Wq, Wk, Wv):
    bf = ml_dtypes.bfloat16
    cmask = _make_cmask()
    in_maps = []
    for c in range(N_CORES):
        b, hg = c // 2, c % 2
        r0 = hg * OUTW
        in_maps.append({
            "xt": np.ascontiguousarray(x[b].T).astype(bf),
            "wqt": np.ascontiguousarray(Wq[r0:r0 + OUTW].T).astype(bf),
            "wkt": np.ascontiguousarray(Wk[r0:r0 + OUTW].T).astype(bf),
            "wvt": np.ascontiguousarray(Wv[r0:r0 + OUTW].T).astype(bf),
            "cmask": cmask,
        })
    return in_maps


def kernel(x, Wq, Wk, Wv):
    from concourse.bass_utils import run_bass_kernel_spmd

    nc = _get_nc()
    in_maps = _make_in_maps(x, Wq, Wk, Wv)
    res = run_bass_kernel_spmd(nc, in_maps, core_ids=list(range(N_CORES)))

    B = x.shape[0]
    out = np.empty((B, T, D), dtype=np.float32)
    for c in range(N_CORES):
        b, hg = c // 2, c % 2
        out[b, :, hg * OUTW:(hg + 1) * OUTW] = res.results[c]["out"]
    return out


# revision 13
# speedup vs baseline: 1.3272x; 1.3272x over previous
"""Multi-head causal attention (B=4, T=2048, D=512, H=8) on 8 TRN2 NeuronCores.

Sharding: core c handles batch b = c//2 and head-group hg = c%2 (4 heads,
256 output dims).  No collectives needed — 8 fully independent problems.

Per-core algorithm (matmul inputs bf16, O^T accumulation f32 in PSUM):
  - host passes x^T (D,T) and W^T slices (D, 256) in bf16 + a [128,128]
    triangular causal mask
  - Q^T,K^T projections:  qT[dh2,T] = W2h @ xT, two heads stacked per tile
  - V projection into augmented-V tiles [k-tile 128, 65] (ones column
    appended -> the O^T matmul also produces the softmax denominator row)
  - flash-style: S^T[k,q] = K^T.T @ Q^T per (k-tile, q-block), exp via ACT
    (scale=1/8 folded in; no max subtraction: |scores| < ~4).  Causal:
    diagonal k-tiles only compute q >= k-tile start, triangle-mask multiply
    on boundary blocks.
  - O^T accumulated in PSUM over k-tiles, then PE-transpose + divide by
    denominator -> natural [T,256] -> DMA out

Scheduling (program order == Tile priority): minimal projection prologue
(K chunks + last Q chunk of group 0), then attention units with the
remaining projection/V units woven between score batches as PE filler so
the exp stream (ACT, the critical engine) starts ~13us in and never
starves.  qb runs DESCENDING so attention opens with its PE-densest
stretch — the PE clock (HAM) never throttles down; re-warming from cold
needs a fully-busy 3.4us window that sparse-qb units can't provide.
"""

import numpy as np
import ml_dtypes

T = 2048
D = 512
HG = 4  # heads per core
DH = 64
OUTW = HG * DH  # 256
QB = 512  # q block (columns of S^T tiles)
NQB = T // QB  # 4
NKT = T // 128  # 16 k-tiles
N_CORES = 8

_CACHE = {}


def _build_nc():
    import concourse.bacc as bacc
    import concourse.tile as tile
    import concourse.mybir as mybir
    from concourse.masks import make_identity
    from contextlib import ExitStack

    fp32 = mybir.dt.float32
    bf16 = mybir.dt.bfloat16
    EXP = mybir.ActivationFunctionType.Exp

    nc = bacc.Bacc(None, target_bir_lowering=False)

    xt_d = nc.declare_dram_parameter("xt", [D, T], bf16, isOutput=False)
    wqt_d = nc.declare_dram_parameter("wqt", [D, OUTW], bf16, isOutput=False)
    wkt_d = nc.declare_dram_parameter("wkt", [D, OUTW], bf16, isOutput=False)
    wvt_d = nc.declare_dram_parameter("wvt", [D, OUTW], bf16, isOutput=False)
    cmask_d = nc.declare_dram_parameter("cmask", [128, 128], bf16, isOutput=False)
    out_d = nc.declare_dram_parameter("out", [T, OUTW], fp32, isOutput=True)

    with tile.TileContext(nc) as tc, ExitStack() as ctx:
        const = ctx.enter_context(tc.tile_pool(name="const", bufs=1))
        ps_s = ctx.enter_context(tc.tile_pool(name="ps_s", bufs=2, space="PSUM"))
        pt_pool = ctx.enter_context(tc.tile_pool(name="pt", bufs=4))
        osb_pool = ctx.enter_context(tc.tile_pool(name="osb", bufs=2))
        rec_pool = ctx.enter_context(tc.tile_pool(name="rec", bufs=4))

        # ---- input loads: weights + x split across both HWDGE queues
        def load4(dram, name, width, engs):
            ts = []
            for c in range(4):
                t = const.tile([128, width], bf16, tag=f"{name}{c}", name=f"{name}{c}")
                engs[c % len(engs)].dma_start(
                    out=t[:], in_=dram[c * 128:(c + 1) * 128, :]
                )
                ts.append(t)
            return ts

        wkT = load4(wkt_d, "wkT", OUTW, [nc.sync])
        wqT = load4(wqt_d, "wqT", OUTW, [nc.sync])
        xT = load4(xt_d, "xT", T, [nc.scalar, nc.sync])
        wvT = load4(wvt_d, "wvT", OUTW, [nc.scalar])

        mask_sb = const.tile([128, 128], bf16, name="mask_sb")
        nc.scalar.dma_start(out=mask_sb[:], in_=cmask_d[:])

        ident = const.tile([128, 128], fp32, name="ident")
        make_identity(nc, ident[:])

        # ---- persistent SBUF tensors ----
        qT = [const.tile([128, T], bf16, tag=f"qT{g}", name=f"qT{g}") for g in range(2)]
        kT = [const.tile([128, T], bf16, tag=f"kT{g}", name=f"kT{g}") for g in range(2)]
        vaug = const.tile([128, NKT, HG, 65], bf16, name="vaug")
        nc.vector.memset(vaug[:, :, :, 64:65], 1.0)
        out_sb = const.tile([128, NQB * 4, OUTW], fp32, tag="out_sb", name="out_sb")

        def proj_qk(dst, wt, g, qb4):
            ps = ps_s.tile([128, QB], fp32, tag="ps", name="ps")
            for c in range(4):
                nc.tensor.matmul(
                    ps[:],
                    wt[c][:, g * 128:(g + 1) * 128],
                    xT[c][:, qb4 * QB:(qb4 + 1) * QB],
                    start=(c == 0),
                    stop=(c == 3),
                )
            nc.vector.tensor_copy(dst[g][:, qb4 * QB:(qb4 + 1) * QB], ps[:])

        def proj_v(tt):
            ps = ps_s.tile([128, OUTW], fp32, tag="ps", name="ps")
            for c in range(4):
                nc.tensor.matmul(
                    ps[:],
                    xT[c][:, tt * 128:(tt + 1) * 128],
                    wvT[c][:, 0:OUTW],
                    start=(c == 0),
                    stop=(c == 3),
                )
            nc.vector.tensor_copy(
                vaug[:, tt, :, 0:64],
                ps[:].rearrange("p (h d) -> p h d", h=HG),
            )

        def attn_unit(qb, h, filler=None):
            """One (head, q-block) attention unit.  filler(i) is called
            between score-batch i and its exp to weave in other PE work."""
            g, po = h // 2, 64 * (h % 2)
            ot = ps_s.tile([128, QB], fp32, tag="ot", name="ot")
            last_kt = qb * 4 + 3

            def score_mm(st_ap, kt, q0, width):
                nc.tensor.matmul(
                    st_ap,
                    kT[g][po:po + 64, kt * 128:(kt + 1) * 128],
                    qT[g][po:po + 64, qb * QB + q0: qb * QB + q0 + width],
                    start=True,
                    stop=True,
                )

            def ot_mm(kt, pt_ap, q0, width):
                nc.tensor.matmul(
                    ot[0:65, q0:q0 + width],
                    vaug[:, kt, h, :],
                    pt_ap,
                    start=(kt == 0),
                    stop=(kt == last_kt),
                )

            # batch list: off-diagonal pairs, then two diagonal pairs with
            # causal restriction (widths 512/384 and 256/128)
            batches = [((kt0, 0, QB), (kt0 + 1, 0, QB))
                       for kt0 in range(0, qb * 4, 2)]
            batches.append(((qb * 4 + 0, 0, QB), (qb * 4 + 1, 128, QB - 128)))
            batches.append(((qb * 4 + 2, 256, QB - 256), (qb * 4 + 3, 384, QB - 384)))

            for i, ((kta, qa, wa), (ktb, qbk, wb)) in enumerate(batches):
                diag = i >= len(batches) - 2
                st = ps_s.tile([128, 2 * QB], fp32, tag="st", name="st")
                score_mm(st[:, 0:wa], kta, qa, wa)
                score_mm(st[:, wa:wa + wb], ktb, qbk, wb)
                if filler:
                    filler(i)
                pt = pt_pool.tile([128, 2 * QB], bf16, tag="pt", name="pt")
                nc.scalar.activation(
                    pt[:, 0:wa + wb], st[:, 0:wa + wb], func=EXP, scale=0.125
                )
                if diag:
                    nc.vector.tensor_mul(pt[:, 0:128], pt[:, 0:128], mask_sb[:])
                    nc.vector.tensor_mul(
                        pt[:, wa:wa + 128], pt[:, wa:wa + 128], mask_sb[:]
                    )
                ot_mm(kta, pt[:, 0:wa], qa, wa)
                ot_mm(ktb, pt[:, wa:wa + wb], qbk, wb)

            # normalize + transpose to natural layout
            osb = osb_pool.tile([65, QB], fp32, tag="osb", name="osb")
            nc.vector.tensor_copy(osb[:], ot[0:65, :])
            for j4 in range(4):
                tp = ps_s.tile([128, 128], fp32, tag="ps", name="tp")
                nc.tensor.transpose(
                    tp[:, 0:65],
                    osb[:, j4 * 128:(j4 + 1) * 128],
                    ident[0:65, 0:65],
                )
                rec = rec_pool.tile([128, 1], fp32, tag="rec", name="rec")
                nc.vector.reciprocal(rec[:], tp[:, 64:65])
                nc.vector.tensor_scalar_mul(
                    out_sb[:, qb * 4 + j4, h * 64:(h + 1) * 64],
                    tp[:, 0:64],
                    rec[:],
                )

        def stream_out(qb):
            for j4 in range(4):
                tt = qb * 4 + j4
                nc.sync.dma_start(
                    out=out_d[tt * 128:(tt + 1) * 128, :], in_=out_sb[:, tt, :]
                )

        # ---- schedule ----
        # minimal prologue: K group-0 fully (scores batch i reads kT chunk
        # i//2 -- too tight to weave) + the one Q chunk attention starts with
        for qb4 in range(4):
            proj_qk(kT, wkT, 0, qb4)
        proj_qk(qT, wqT, 0, 3)

        # filler plans: V units into (3,h0) (ot batch i needs vaug 2i,2i+1);
        # group-1 K/Q + remaining group-0 Q chunks into (3,h1)
        fill_h0 = {i: [("v", 2 * i), ("v", 2 * i + 1)] for i in range(8)}
        fill_h1 = {
            0: [("k", 1, 0), ("k", 1, 1)],
            1: [("k", 1, 2), ("k", 1, 3)],
            2: [("q", 1, 3), ("q", 1, 2)],
            3: [("q", 1, 1)],
            4: [("q", 1, 0)],
            5: [("q", 0, 2)],
            6: [("q", 0, 1)],
            7: [("q", 0, 0)],
        }

        def run_fill(plan, i):
            for item in plan.get(i, ()):
                if item[0] == "v":
                    proj_v(item[1])
                elif item[0] == "k":
                    proj_qk(kT, wkT, item[1], item[2])
                else:
                    proj_qk(qT, wqT, item[1], item[2])

        attn_unit(3, 0, filler=lambda i: run_fill(fill_h0, i))
        attn_unit(3, 1, filler=lambda i: run_fill(fill_h1, i))
        attn_unit(3, 2)
        attn_unit(3, 3)
        stream_out(3)
        for qb in (2, 1, 0):
            for h in range(HG):
                attn_unit(qb, h)
            stream_out(qb)

    nc.finalize()
    return nc


def _get_nc():
    if "nc" not in _CACHE:
        _CACHE["nc"] = _build_nc()
    return _CACHE["nc"]


def _make_cmask():
    # triangle: mask[p, f] = 1.0 iff p <= f
    p = np.arange(128)[:, None]
    f = np.arange(128)[None, :]
    return (p <= f).astype(ml_dtypes.bfloat16)


def _make_in_maps(x, Wq, Wk, Wv):
    bf = ml_dtypes.bfloat16
    cmask = _make_cmask()
    in_maps = []
    for c in range(N_CORES):
        b, hg = c // 2, c % 2
        r0 = hg * OUTW
        in_maps.append({
            "xt": np.ascontiguousarray(x[b].T).astype(bf),
            "wqt": np.ascontiguousarray(Wq[r0:r0 + OUTW].T).astype(bf),
            "wkt": np.ascontiguousarray(Wk[r0:r0 + OUTW].T).astype(bf),
            "wvt": np.ascontiguousarray(Wv[r0:r0 + OUTW].T).astype(bf),
            "cmask": cmask,
        })
    return in_maps


def kernel(x, Wq, Wk, Wv):
    from concourse.bass_utils import run_bass_kernel_spmd

    nc = _get_nc()
    in_maps = _make_in_maps(x, Wq, Wk, Wv)
    res = run_bass_kernel_spmd(nc, in_maps, core_ids=list(range(N_CORES)))

    B = x.shape[0]
    out = np.empty((B, T, D), dtype=np.float32)
    for c in range(N_CORES):
        b, hg = c // 2, c % 2
        out[b, :, hg * OUTW:(hg + 1) * OUTW] = res.results[c]["out"]
    return out
